# revision 1
# baseline (speedup 1.0000x reference)
"""DiffTreeInterpreter scatter-coalesce kernel for 8 Trainium2 cores.

Data-parallel over batch B=32: core c owns batches [4c, 4c+4). All
scatter-adds are device-local. Host work is limited to sharding-style
index prep: bucketing entries by (batch, role-block), and shipping
bit-exact *copies* of per-entry weights (arg_weights / op_dist rows
selected by index) alongside the value stream. All arithmetic
(weight products, value scaling, coalesce sums, stream combine)
happens on the NeuronCores.

Math (see reference): with H = R/2, each entry n (b, l, r, v=mem[n],
w=arg_weights[b,l]) contributes to out[b] at up to 3 bins:
  bin r>>1   with weight op0[b]*w0 if r even, op1[b]*w1 if r odd and r!=1
  bin 2r     with weight op2[b]*w2 (only r < H)
  bin 2r+1   with weight op2[b]*w3 (only r < H)
plus out[b,1] += op2[b]*root_filler[b].
(The reference's pad-mask is a no-op on values: masked rows are all-zero.)

Device algorithm per core: entries are bucketed into 128-entry tiles
aligned to role windows; tiles are organized into 16 groups per batch
(8 "lower" groups of 8 tiles covering r<2048, feeding both the
car/cdr stream and the interleaved cons stream; 8 "upper" groups of
5 tiles covering r>=2048, car/cdr only). Per group, GPSIMD
local_scatter builds u-scaled one-hot slabs in fp16 (u = weight
products computed on the Vector engine); the PE contracts one-hot^T @
values into PSUM blocks of 128 output bins; PSUM drains into a
per-batch SBUF output region (ACT copies + DVE adds) which is written
out in chunks as bin-blocks finalize.

Measured on 8 trn2 cores: ~102 us HW exec, rel err ~3.4e-4 (fp16
matmul operands; fp32 accumulation and output).
"""

import sys

if "/opt/trn_rl_repo" not in sys.path:
    sys.path.insert(0, "/opt/trn_rl_repo")

import numpy as np

B, L, F, R = 32, 128, 128, 4096
H = R >> 1
N = 262144
NCORES = 8
BPC = B // NCORES  # batches per core

P = 128  # partitions / tile entry count / bin-block size

# Static schedule per batch: 16 groups covering 256 roles each; lower
# groups g<8 (r<2048) hold 2 pairs of cons blocks, straddle-packed as
# 3 tiles per pair (T0 pure-A, T1 = A-overflow + B-overflow, T2
# pure-B); upper groups 5 tiles, car/cdr only.
NG = 16
LOW_TPG = 6   # tiles per lower group (2 pairs x 3)
UP_CAP = 5    # tiles per (batch, 256-r block); holds <= 640 entries
BLK_CAP = 256   # max entries per 64-r cons block
PAIR_CAP = 384  # max entries per cons block pair
TILES_PER_BATCH = 8 * LOW_TPG + 8 * UP_CAP  # 88
NSLOT = NG * 8  # group-padded slot space
NT = BPC * TILES_PER_BATCH  # tiles per core (352)

# meta channels (fp32, slot space)
MC_WA, MC_OPA, MC_WB, MC_WC, MC_OP2, MC_R1, MC_R23, MC_PAD = range(8)
NMC = 8

_PROG_CACHE = {}

CONFIG = {
    "val_dtype": "float16",  # PE operand dtype (values + one-hots)
    "vload_batch": 8,        # value tiles per load DMA
}


def _slot_of(g, tloc):
    return g * 8 + tloc


def _tile_of(g, tloc):
    if g < 8:
        return g * LOW_TPG + tloc
    return 8 * LOW_TPG + (g - 8) * UP_CAP + tloc


def _build_program():
    import concourse.bacc as bacc
    import concourse.mybir as mybir
    import concourse.tile as tile

    fp32 = mybir.dt.float32
    i16 = mybir.dt.int16
    vdt = getattr(mybir.dt, CONFIG["val_dtype"])
    MUL = mybir.AluOpType.mult
    ADD = mybir.AluOpType.add
    EQ = mybir.AluOpType.is_equal
    VB = CONFIG["vload_batch"]
    assert TILES_PER_BATCH % VB == 0

    nc = bacc.Bacc(None, target_bir_lowering=False)
    # values grouped by load-slab: [group, partition, tile-in-group, F] so
    # each partition's DMA read is VB*F contiguous elements
    vals = nc.dram_tensor("vals", [NT // VB, P, VB, F], vdt,
                          kind="ExternalInput")
    meta = nc.dram_tensor("meta", [BPC, P, NSLOT, NMC], fp32,
                          kind="ExternalInput")
    idx1 = nc.dram_tensor("idx1", [BPC, P, NG, 8], i16, kind="ExternalInput")
    idx23 = nc.dram_tensor("idx23", [BPC, P, 8, 16], i16, kind="ExternalInput")
    iota = nc.dram_tensor("iota", [P, P], fp32, kind="ExternalInput")
    out = nc.dram_tensor("out", [BPC, R, F], fp32, kind="ExternalOutput")

    with tile.TileContext(nc) as tc:
        with tc.tile_pool(name="metap", bufs=BPC) as mpool, \
             tc.tile_pool(name="useq", bufs=BPC) as upool, \
             tc.tile_pool(name="vload", bufs=12) as vpool, \
             tc.tile_pool(name="ohot", bufs=10) as opool, \
             tc.tile_pool(name="outreg", bufs=2) as rpool, \
             tc.tile_pool(name="ps1", bufs=4, space="PSUM") as ps1pool, \
             tc.tile_pool(name="ps23", bufs=4, space="PSUM") as ps23pool:

            vtiles = {}

            io_t = mpool.tile([P, P], fp32, tag="iota")
            nc.sync.dma_start(out=io_t[:], in_=iota[:])

            def vload_group(gidx):
                if gidx not in vtiles:
                    vt = vpool.tile([P, VB, F], vdt, tag="v")
                    nc.sync.dma_start(out=vt[:], in_=vals[gidx])
                    vtiles[gidx] = vt

            # prefetch all batches' metadata up front (small, keeps the
            # batch-transition critical path off the DMA queue); the first
            # value slabs go ahead of later batches' metadata
            metas = []
            for b in range(BPC):
                m = mpool.tile([P, NSLOT, NMC], fp32, tag="m")
                nc.sync.dma_start(out=m[:], in_=meta[b])
                x1 = mpool.tile([P, NG, 8], i16, tag="x1")
                nc.sync.dma_start(out=x1[:], in_=idx1[b])
                x23 = mpool.tile([P, 8, 16], i16, tag="x23")
                nc.sync.dma_start(out=x23[:], in_=idx23[b])
                u1 = upool.tile([P, NSLOT], vdt, tag="u1")
                nc.vector.tensor_tensor(
                    out=u1[:], in0=m[:, :, MC_WA], in1=m[:, :, MC_OPA], op=MUL)
                u1f = upool.tile([P, NSLOT], fp32, tag="u1f")
                nc.vector.tensor_tensor(
                    out=u1f[:], in0=m[:, :, MC_WA], in1=m[:, :, MC_OPA], op=MUL)
                # cons u slab [P, 8 groups, 16]: u2 in cols 0:8, u3 in 8:16
                u23 = upool.tile([P, 8, 16], vdt, tag="u23")
                lo_slots = m[:, 0:64, :].rearrange("p (g t) c -> p g t c", t=8)
                nc.vector.tensor_tensor(
                    out=u23[:, :, 0:8], in0=lo_slots[:, :, :, MC_WB],
                    in1=lo_slots[:, :, :, MC_OP2], op=MUL)
                nc.vector.tensor_tensor(
                    out=u23[:, :, 8:16], in0=lo_slots[:, :, :, MC_WC],
                    in1=lo_slots[:, :, :, MC_OP2], op=MUL)
                metas.append((m, x1, x23, u1, u1f, u23))
                if b == 0:
                    for gidx in range(3):
                        vload_group(gidx)

            for b in range(BPC):
                m, x1, x23, u1, u1f, u23 = metas[b]
                outreg = rpool.tile([P, 32 * P], fp32)

                def vtile(t):
                    tg = b * TILES_PER_BATCH + t
                    vload_group(tg // VB)
                    return vtiles[tg // VB][:, tg % VB, :]

                for g in range(NG):
                    lower = g < 8
                    ntiles = LOW_TPG if lower else UP_CAP
                    ps1 = ps1pool.tile([P, F], fp32, tag="ps1")
                    # group one-hot slabs: GPSIMD local scatter, except a
                    # share of upper groups built per-tile on the Vector
                    # engine to balance the two
                    o1s = opool.tile([P, 8 * P], vdt, tag="o1s")
                    if lower or g == 8:
                        nc.gpsimd.local_scatter(
                            out_ap=o1s[:, :ntiles * P],
                            data_ap=u1[:, g * 8:g * 8 + 8],
                            idxs_ap=x1[:, g, :],
                            channels=P, num_elems=ntiles * P, num_idxs=8)
                    else:
                        for tloc in range(ntiles):
                            s = g * 8 + tloc
                            nc.vector.tensor_scalar(
                                out=o1s[:, tloc * P:(tloc + 1) * P],
                                in0=io_t[:],
                                scalar1=m[:, s, MC_R1:MC_R1 + 1],
                                scalar2=u1f[:, s:s + 1],
                                op0=EQ, op1=MUL)
                    if lower:
                        # cons one-hot ranges: per pair q, 4 ranges of 128
                        # cols: [T0->blkA, T1A->blkA, T1B->blkB, T2->blkB]
                        o23s = opool.tile([P, 8 * P], vdt, tag="o23s")
                        nc.gpsimd.local_scatter(
                            out_ap=o23s[:], data_ap=u23[:, g, :],
                            idxs_ap=x23[:, g, :],
                            channels=P, num_elems=8 * P, num_idxs=16)
                    if lower:
                        i1 = 0
                        for q in range(2):
                            tau = 3 * q
                            vv = [vtile(_tile_of(g, tau + j)) for j in range(3)]
                            psA = ps23pool.tile([P, F], fp32, tag="ps23")
                            nc.tensor.matmul(
                                out=ps1[:], lhsT=o1s[:, tau * P:(tau + 1) * P],
                                rhs=vv[0], start=(i1 == 0), stop=False)
                            i1 += 1
                            nc.tensor.matmul(
                                out=psA[:], lhsT=o23s[:, (4 * q) * P:(4 * q + 1) * P],
                                rhs=vv[0], start=True, stop=False)
                            nc.tensor.matmul(
                                out=ps1[:], lhsT=o1s[:, (tau + 1) * P:(tau + 2) * P],
                                rhs=vv[1], start=False, stop=False)
                            i1 += 1
                            nc.tensor.matmul(
                                out=psA[:], lhsT=o23s[:, (4 * q + 1) * P:(4 * q + 2) * P],
                                rhs=vv[1], start=False, stop=True)
                            kA = 4 * g + 2 * q
                            nc.scalar.copy(
                                out=outreg[:, kA * P:(kA + 1) * P], in_=psA[:])
                            psB = ps23pool.tile([P, F], fp32, tag="ps23")
                            nc.tensor.matmul(
                                out=psB[:], lhsT=o23s[:, (4 * q + 2) * P:(4 * q + 3) * P],
                                rhs=vv[1], start=True, stop=False)
                            nc.tensor.matmul(
                                out=ps1[:], lhsT=o1s[:, (tau + 2) * P:(tau + 3) * P],
                                rhs=vv[2], start=False, stop=(i1 == LOW_TPG - 1))
                            i1 += 1
                            nc.tensor.matmul(
                                out=psB[:], lhsT=o23s[:, (4 * q + 3) * P:(4 * q + 4) * P],
                                rhs=vv[2], start=False, stop=True)
                            kB = kA + 1
                            nc.scalar.copy(
                                out=outreg[:, kB * P:(kB + 1) * P], in_=psB[:])
                    else:
                        for tloc in range(ntiles):
                            v = vtile(_tile_of(g, tloc))
                            nc.tensor.matmul(
                                out=ps1[:], lhsT=o1s[:, tloc * P:(tloc + 1) * P],
                                rhs=v, start=(tloc == 0),
                                stop=(tloc == ntiles - 1))
                    # car/cdr drain: bins [128g, +128) add onto cons copy
                    nc.vector.tensor_tensor(
                        out=outreg[:, g * P:(g + 1) * P],
                        in0=outreg[:, g * P:(g + 1) * P], in1=ps1[:], op=ADD)

                    # flush finished bin-blocks early to shorten the tail:
                    # after g7, blocks 0-7 (car/cdr done) and 16-31 (cons
                    # only) are final; blocks 8-15 finalize at their group.
                    def flush(k0, k1):
                        nc.sync.dma_start(
                            out=out[b, k0 * P:k1 * P, :]
                            .rearrange("(k p) f -> p k f", p=P),
                            in_=outreg[:, k0 * P:k1 * P]
                            .rearrange("p (k f) -> p k f", f=F))
                    if g == 7:
                        flush(0, 8)
                        flush(16, 32)
                    elif g == 11:
                        flush(8, 12)
                    elif g == 15:
                        flush(12, 16)

    nc.compile()
    return nc


def _pack_inputs(mem_values, arg_weights, root_filler, op_dist,
                 batch_idx, slot_idx, role_idx):
    """Host-side sharding/packing. Index selection and copies only."""
    mem_values = np.ascontiguousarray(mem_values, dtype=np.float32)
    arg_weights = np.asarray(arg_weights, dtype=np.float32)
    root_filler = np.asarray(root_filler, dtype=np.float32)
    op_dist = np.asarray(op_dist, dtype=np.float32)
    batch_idx = np.asarray(batch_idx, dtype=np.int64)
    slot_idx = np.asarray(slot_idx, dtype=np.int64)
    role_idx = np.asarray(role_idx, dtype=np.int64)

    # per-entry selected copies (pure gathers, no arithmetic)
    w = arg_weights[batch_idx, slot_idx]  # [N, 4] copies
    r = role_idx
    even = (r & 1) == 0
    wA = np.where(even, w[:, 0], np.where(r != 1, w[:, 1], 0.0)).astype(np.float32)
    opA = np.where(even, op_dist[batch_idx, 0],
                   op_dist[batch_idx, 1]).astype(np.float32)
    lo = r < H
    wB = np.where(lo, w[:, 2], 0.0).astype(np.float32)
    wC = np.where(lo, w[:, 3], 0.0).astype(np.float32)
    op2c = op_dist[batch_idx, 2].astype(np.float32)

    # block id within batch: lower cons blocks 0..31 (64 r each),
    # upper blocks 32..39 (256 r each)
    blk = np.where(lo, r >> 6, 32 + ((r - H) >> 8))

    vdt = np.dtype(CONFIG["val_dtype"])
    VB = CONFIG["vload_batch"]
    in_maps = []
    for c in range(NCORES):
        vals_s = np.zeros((NT * P, F), vdt)
        # entry-indexed (tile space) scratch, converted to slot space below
        r1_rel = np.full((NT, P), -1, np.int64)
        r23_rel = np.full((NT, P), -1, np.int64)
        wA_t = np.zeros((NT, P), np.float32)
        opA_t = np.zeros((NT, P), np.float32)
        wB_t = np.zeros((NT, P), np.float32)
        wC_t = np.zeros((NT, P), np.float32)
        op2_t = np.zeros((NT, P), np.float32)
        rho_t = np.full((NT, P), -1, np.int64)  # cons col-range per entry
        for bb in range(BPC):
            b = c * BPC + bb
            sel = np.nonzero(batch_idx == b)[0]
            gb = blk[sel]
            order = np.argsort(gb, kind="stable")
            sel = sel[order]
            gb = gb[order]
            counts = np.bincount(gb, minlength=40)
            counts_root = counts.copy()
            counts_root[0] += 1  # synthetic root entry joins block 0
            pair_sum = counts_root[:32].reshape(16, 2).sum(1)
            if (counts_root[:32] > BLK_CAP).any() or \
               (pair_sum > PAIR_CAP).any() or \
               (counts_root[32:] > UP_CAP * P).any():
                raise RuntimeError(
                    "static schedule capacity exceeded: "
                    f"lower={counts_root[:32].max()} pair={pair_sum.max()} "
                    f"upper={counts_root[32:].max()}")
            first = np.concatenate([[0], np.cumsum(counts)])[:-1]
            pos = np.arange(sel.size) - first[gb]

            def place(gbv, posv):
                """(block, pos-in-block) -> (tile-in-batch, partition,
                cons col-range rho or -1). Lower pairs straddle-packed:
                T0 pure-A, T2 pure-B, T1 = A overflow then B overflow."""
                low = gbv < 32
                gg = gbv >> 2
                qq = (gbv >> 1) & 1
                side = gbv & 1
                ov = posv >= P
                cA = counts_root[np.clip(gbv & ~1, 0, 39)]
                cAover = np.maximum(cA - P, 0)
                tau_lo = np.where(ov, 3 * qq + 1,
                                  np.where(side == 0, 3 * qq, 3 * qq + 2))
                part_lo = np.where(~ov, posv,
                                   np.where(side == 0, posv - P,
                                            cAover + posv - P))
                rho_lo = 4 * qq + np.where(
                    ov, np.where(side == 0, 1, 2),
                    np.where(side == 0, 0, 3))
                tile_lo = gg * LOW_TPG + tau_lo
                ug = gbv - 32
                tile_up = 8 * LOW_TPG + ug * UP_CAP + posv // P
                tile = np.where(low, tile_lo, tile_up)
                part = np.where(low, part_lo, posv % P)
                rho = np.where(low, rho_lo, -1)
                return tile, part, rho

            tile_a, part_a, rho_a = place(gb, pos)
            tix = bb * TILES_PER_BATCH + tile_a
            pix = part_a
            vals_s[tix * P + pix] = mem_values[sel]
            rr = role_idx[sel]
            r1_rel[tix, pix] = (rr >> 1) & 127
            r23_rel[tix, pix] = np.where(rr < H, rr & 63, -1)
            rho_t[tix, pix] = rho_a
            wA_t[tix, pix] = wA[sel]
            opA_t[tix, pix] = opA[sel]
            wB_t[tix, pix] = wB[sel]
            wC_t[tix, pix] = wC[sel]
            op2_t[tix, pix] = op2c[sel]
            # synthetic root entry -> bin 1 == 2*0+1 (block 0, odd cons)
            rt, rp, rrho = place(np.array([0]), np.array([counts[0]]))
            ti = bb * TILES_PER_BATCH + rt[0]
            pi = rp[0]
            vals_s[ti * P + pi] = root_filler[b]
            r1_rel[ti, pi] = -1
            r23_rel[ti, pi] = 0
            rho_t[ti, pi] = rrho[0]
            wC_t[ti, pi] = 1.0
            op2_t[ti, pi] = op_dist[b, 2]

        # tile space -> slot space
        meta_s = np.zeros((BPC, NSLOT, P, NMC), np.float32)
        idx1_s = np.full((BPC, NG, P, 8), -1, np.int16)
        idx23_s = np.full((BPC, 8, P, 16), -1, np.int16)
        for bb in range(BPC):
            for g in range(NG):
                ntl = LOW_TPG if g < 8 else UP_CAP
                for tloc in range(ntl):
                    t = bb * TILES_PER_BATCH + _tile_of(g, tloc)
                    s = _slot_of(g, tloc)
                    meta_s[bb, s, :, MC_WA] = wA_t[t]
                    meta_s[bb, s, :, MC_OPA] = opA_t[t]
                    meta_s[bb, s, :, MC_WB] = wB_t[t]
                    meta_s[bb, s, :, MC_WC] = wC_t[t]
                    meta_s[bb, s, :, MC_OP2] = op2_t[t]
                    meta_s[bb, s, :, MC_R1] = r1_rel[t]
                    meta_s[bb, s, :, MC_R23] = r23_rel[t]
                    v1 = r1_rel[t] >= 0
                    idx1_s[bb, g, :, tloc] = np.where(
                        v1, tloc * P + r1_rel[t], -1)
                    if g < 8:
                        v23 = r23_rel[t] >= 0
                        base = rho_t[t] * P + 2 * r23_rel[t]
                        idx23_s[bb, g, :, tloc] = np.where(v23, base, -1)
                        idx23_s[bb, g, :, 8 + tloc] = np.where(v23, base + 1, -1)

        in_maps.append({
            # [NT*P, F] -> [NT//VB, P, VB, F] load-grouped layout
            "vals": np.ascontiguousarray(
                vals_s.reshape(NT // VB, VB, P, F).transpose(0, 2, 1, 3)),
            # partition-major layouts so each partition's DMA is contiguous
            "meta": np.ascontiguousarray(meta_s.transpose(0, 2, 1, 3)),
            "idx1": np.ascontiguousarray(idx1_s.transpose(0, 2, 1, 3)),
            "idx23": np.ascontiguousarray(idx23_s.transpose(0, 2, 1, 3)),
            "iota": np.broadcast_to(
                np.arange(P, dtype=np.float32), (P, P)).copy(),
        })
    return in_maps


def kernel(**inputs):
    from concourse.bass_utils import run_bass_kernel_spmd

    in_maps = _pack_inputs(**inputs)
    if "nc" not in _PROG_CACHE:
        _PROG_CACHE["nc"] = _build_program()
    nc = _PROG_CACHE["nc"]
    res = run_bass_kernel_spmd(nc, in_maps, list(range(NCORES)))
    return np.concatenate([res.results[c]["out"] for c in range(NCORES)], axis=0)



# revision 7
# speedup vs baseline: 1.1452x; 1.1452x over previous
"""DiffTreeInterpreter scatter-coalesce kernel for 8 Trainium2 cores.

Data-parallel over batch B=32: core c owns batches [4c, 4c+4). Host work
is index prep only: bucketing entries by (batch, role-group), shipping
bit-exact copies of per-entry weights; all arithmetic happens on device.

Math (see reference): with H = R/2, entry n (b, l, r, v=mem[n],
w=arg_weights[b,l]) contributes to out[b] at up to 3 bins:
  bin r>>1  with uA = op0*w0 (r even) / op1*w1 (r odd, r!=1)   [A-stream]
  bin 2r    with u2 = op2*w2  (only r < H)                     [cons even]
  bin 2r+1  with u3 = op2*w3  (only r < H)                     [cons odd]
plus out[b,1] += op2*root_filler[b]. (Pad-mask is a no-op: masked rows
are all-zero values.)

Device design (per batch):
- 16 groups of 256 roles, 5 value tiles (128 entries) each = 80 tiles.
  Lower groups g<8 chain-pack their 4 cons blocks (64 roles) into the 5
  tiles with static (tile, block) incidence T0:A T1:AB T2:BC T3:CD T4:D.
- One-hot slabs [entry-partition, cols] fp16: lower = merged [640 A-cols
  | 1024 cons-cols] built by one GPSIMD local_scatter; upper A-slabs
  [640] built per-tile on DVE (iota EQ r1 * u) or ACT (2-op tent:
  Square then Relu with per-partition bias/scale).
- Matmuls are value-stationary: out[f, bins] = v[entry,F]^T @ slab.
  PSUM holds 8 "superblock" banks [128, 512 bins] per batch; A-stream
  and cons matmuls accumulate into shared bank slices (A starts a
  slice, cons of the owning lower group finishes it), so no separate
  combine pass exists.
- Banks drain via one wide ACT copy [128,512] into paired staging, then
  DMA to out[b, F, R] (host transposes at unshard).
"""

import sys

if "/opt/trn_rl_repo" not in sys.path:
    sys.path.insert(0, "/opt/trn_rl_repo")

import numpy as np

B, L, F, R = 32, 128, 128, 4096
H = R >> 1
N = 262144
NCORES = 8
BPC = B // NCORES
P = 128

NG = 16          # role groups per batch (256 roles each)
TPG = 5          # value tiles per group
TILES_PER_BATCH = NG * TPG  # 80
NT = BPC * TILES_PER_BATCH  # 320 tiles per core
VB = 16          # value tiles per load DMA
NVS = NT // VB   # 20 value load slabs per core

SLAB_A = TPG * P        # 640
SLAB_W = SLAB_A + 8 * P  # 1664 (A cols + 8 cons ranges)

# upper-group one-hot builder assignment (groups 8..15)
DVE_G = (8, 9, 10, 11, 12, 13)
ACT_G = (14, 15)

_PROG_CACHE = {}


def _build_program():
    import concourse.bacc as bacc
    import concourse.mybir as mybir
    import concourse.tile as tile

    fp32 = mybir.dt.float32
    fp16 = mybir.dt.float16
    i16 = mybir.dt.int16
    MUL = mybir.AluOpType.mult
    EQ = mybir.AluOpType.is_equal
    AF = mybir.ActivationFunctionType

    nc = bacc.Bacc(None, target_bir_lowering=False)
    vals = nc.dram_tensor("vals", [NVS, P, VB, F], fp16, kind="ExternalInput")
    # fp16 meta: [0:160) = (WA, OPA) per slot (g*5+t); [160:280) = lower
    # (WB, WC, OP2) per (g, t)
    meta = nc.dram_tensor("meta", [BPC, P, 280], fp16, kind="ExternalInput")
    # fp32 (r1, -r1) per upper slot ((g-8)*5+t)
    r1pm = nc.dram_tensor("r1pm", [BPC, P, 40, 2], fp32, kind="ExternalInput")
    idxs = nc.dram_tensor("idxs", [BPC, P, NG, 16], i16, kind="ExternalInput")
    iota = nc.dram_tensor("iota", [P, P], fp16, kind="ExternalInput")
    out = nc.dram_tensor("out", [BPC, F, R], fp32, kind="ExternalOutput")

    with tile.TileContext(nc) as tc:
        with tc.tile_pool(name="cst", bufs=1) as cpool, \
             tc.tile_pool(name="meta", bufs=BPC) as mpool, \
             tc.tile_pool(name="ud", bufs=BPC) as upool, \
             tc.tile_pool(name="vload", bufs=10) as vpool, \
             tc.tile_pool(name="slab", bufs=13) as spool, \
             tc.tile_pool(name="sq", bufs=2) as qpool, \
             tc.tile_pool(name="stage", bufs=8) as gpool, \
             tc.tile_pool(name="bank", bufs=8, space="PSUM") as bpool:

            io_t = cpool.tile([P, P], fp16, tag="iota")
            nc.sync.dma_start(out=io_t[:], in_=iota[:])

            vtiles = {}

            def vload(vs):
                if vs not in vtiles:
                    vt = vpool.tile([P, VB, F], fp16, tag="v")
                    nc.sync.dma_start(out=vt[:], in_=vals[vs])
                    vtiles[vs] = vt

            # prefetch metadata for all batches + compute u products
            metas = []
            for b in range(BPC):
                m = mpool.tile([P, 280], fp16, tag="m")
                nc.sync.dma_start(out=m[:], in_=meta[b])
                rp = mpool.tile([P, 40, 2], fp32, tag="rp")
                nc.sync.dma_start(out=rp[:], in_=r1pm[b])
                x = mpool.tile([P, NG, 16], i16, tag="x")
                nc.sync.dma_start(out=x[:], in_=idxs[b])

                m1 = m[:, 0:160].rearrange("p (s c) -> p s c", c=2)
                m2 = m[:, 160:280].rearrange("p (g t c) -> p g t c", g=8, c=3)
                ud = upool.tile([P, NG, 16], fp16, tag="ud")
                # uA = WA*OPA into ud[:, :, 0:5]
                nc.vector.tensor_tensor(
                    out=ud[:, :, 0:5],
                    in0=m1[:, :, 0].rearrange("p (g t) -> p g t", t=TPG),
                    in1=m1[:, :, 1].rearrange("p (g t) -> p g t", t=TPG),
                    op=MUL)
                # u2 = WB*OP2 into ud[:, 0:8, 5:10]; u3 = WC*OP2 into [10:15]
                nc.vector.tensor_tensor(
                    out=ud[:, 0:8, 5:10], in0=m2[:, :, :, 0],
                    in1=m2[:, :, :, 2], op=MUL)
                nc.vector.tensor_tensor(
                    out=ud[:, 0:8, 10:15], in0=m2[:, :, :, 1],
                    in1=m2[:, :, :, 2], op=MUL)
                # fp32 u (and -u) for upper-slot scalar operands
                u1f = upool.tile([P, 40], fp32, tag="u1f")
                nc.vector.tensor_tensor(
                    out=u1f[:], in0=m1[:, 40:80, 0], in1=m1[:, 40:80, 1],
                    op=MUL)
                ngu = upool.tile([P, 40], fp32, tag="ngu")
                nc.vector.tensor_scalar(
                    out=ngu[:], in0=u1f[:], scalar1=-1.0, scalar2=None,
                    op0=MUL)
                metas.append((ud, x, u1f, ngu, rp))
                if b == 0:
                    vload(0)
                    vload(1)

            for b in range(BPC):
                ud, x, u1f, ngu, rp = metas[b]

                def vtile(g, tl):
                    t = b * TILES_PER_BATCH + g * TPG + tl
                    vload(t // VB)
                    if (t // VB) + 1 < NVS:
                        vload(t // VB + 1)
                    return vtiles[t // VB][:, t % VB, :]

                banks = [None] * 8
                slabs = [None] * NG
                stages = [None] * 4

                def drain(k):
                    # pair (k, k^1) shares a staging tile; DMA on 2nd drain
                    pair = k >> 1
                    if stages[pair] is None:
                        stages[pair] = gpool.tile([P, 1024], fp32, tag="st", name="st")
                    st = stages[pair]
                    half = (k & 1) * 512
                    nc.scalar.copy(out=st[:, half:half + 512],
                                   in_=banks[k][:])
                    if k & 1:
                        nc.sync.dma_start(
                            out=out[b, :, (pair * 1024):(pair * 1024 + 1024)],
                            in_=st[:])

                for g in range(NG):
                    # --- build slab(g) ---
                    sl = spool.tile([P, SLAB_W], fp16, tag="sl")
                    slabs[g] = sl
                    if g < 8:
                        nc.gpsimd.local_scatter(
                            out_ap=sl[:], data_ap=ud[:, g, :],
                            idxs_ap=x[:, g, :],
                            channels=P, num_elems=SLAB_W, num_idxs=16)
                    elif g in DVE_G:
                        for tl in range(TPG):
                            s = (g - 8) * TPG + tl
                            nc.vector.tensor_scalar(
                                out=sl[:, tl * P:(tl + 1) * P], in0=io_t[:],
                                scalar1=rp[:, s, 0:1],
                                scalar2=u1f[:, s:s + 1],
                                op0=EQ, op1=MUL)
                    else:
                        for tl in range(TPG):
                            s = (g - 8) * TPG + tl
                            sq = qpool.tile([P, P], fp16, tag="sq")
                            nc.scalar.activation(
                                out=sq[:], in_=io_t[:], func=AF.Square,
                                bias=rp[:, s, 1:2], scale=1.0)
                            nc.scalar.activation(
                                out=sl[:, tl * P:(tl + 1) * P], in_=sq[:],
                                func=AF.Relu, bias=u1f[:, s:s + 1],
                                scale=ngu[:, s:s + 1])

                    # --- A-stream matmuls: bank g>>2, slice g&3 ---
                    # PSUM start=True clears has_written for the WHOLE
                    # bank, so only the bank's first-ever matmul may set
                    # it; start=False overwrites where the bit is clear
                    # and accumulates where set.
                    bk = g >> 2
                    first = banks[bk] is None
                    if first:
                        banks[bk] = bpool.tile([P, 512], fp32, tag="bk", name="bk")
                    sli = (g & 3) * P
                    for tl in range(TPG):
                        v = vtile(g, tl)
                        nc.tensor.matmul(
                            out=banks[bk][:, sli:sli + P], lhsT=v,
                            rhs=sl[:, tl * P:(tl + 1) * P],
                            start=(first and tl == 0), stop=False,
                            skip_group_check=True)

                    # --- cons matmuls ---
                    def cons(cg, bank_fresh):
                        # group cg's cons into bank cg; block j gets
                        # (T_j: range 2j, T_{j+1}: range 2j+1). Only the
                        # bank's first-ever matmul sets start (whole-bank
                        # has_written clear).
                        csl = slabs[cg]
                        for tl in range(TPG):
                            v = vtile(cg, tl)
                            if tl >= 1:  # tile ends block tl-1
                                rng = SLAB_A + (2 * (tl - 1) + 1) * P
                                nc.tensor.matmul(
                                    out=banks[cg][:, (tl - 1) * P:tl * P],
                                    lhsT=v, rhs=csl[:, rng:rng + P],
                                    start=False, stop=True,
                                    skip_group_check=True)
                            if tl <= 3:  # tile starts block tl
                                rng = SLAB_A + (2 * tl) * P
                                nc.tensor.matmul(
                                    out=banks[cg][:, tl * P:(tl + 1) * P],
                                    lhsT=v, rhs=csl[:, rng:rng + P],
                                    start=(bank_fresh and tl == 0),
                                    stop=False, skip_group_check=True)

                    if 4 <= g < 8:
                        if banks[g] is None:
                            banks[g] = bpool.tile([P, 512], fp32, tag="bk", name="bk")
                        cons(g, bank_fresh=True)
                        drain(g)
                    if (g & 3) == 3:
                        cons(g >> 2, bank_fresh=False)
                        drain(g >> 2)

    nc.compile()
    return nc


def _pack_inputs(mem_values, arg_weights, root_filler, op_dist,
                 batch_idx, slot_idx, role_idx):
    """Host-side sharding/packing: index selection and copies only."""
    mem_values = np.ascontiguousarray(mem_values, dtype=np.float32)
    arg_weights = np.asarray(arg_weights, dtype=np.float32)
    root_filler = np.asarray(root_filler, dtype=np.float32)
    op_dist = np.asarray(op_dist, dtype=np.float32)
    batch_idx = np.asarray(batch_idx, dtype=np.int64)
    slot_idx = np.asarray(slot_idx, dtype=np.int64)
    role_idx = np.asarray(role_idx, dtype=np.int64)

    w = arg_weights[batch_idx, slot_idx]  # [N, 4] gathered copies
    r = role_idx
    even = (r & 1) == 0
    wA = np.where(even, w[:, 0], np.where(r != 1, w[:, 1], 0.0))
    opA = np.where(even, op_dist[batch_idx, 0], op_dist[batch_idx, 1])

    iota_np = np.broadcast_to(
        np.arange(P, dtype=np.float16), (P, P)).copy()

    in_maps = []
    for c in range(NCORES):
        vals_s = np.zeros((NT * P, F), np.float16)
        meta_s = np.zeros((BPC, P, 280), np.float16)
        r1pm_s = np.zeros((BPC, P, 40, 2), np.float32)
        r1pm_s[:, :, :, 0] = -1.0
        r1pm_s[:, :, :, 1] = 1.0
        idx_s = np.full((BPC, P, NG, 16), -1, np.int16)

        for bb in range(BPC):
            b = c * BPC + bb
            sel0 = np.nonzero(batch_idx == b)[0]
            rr0 = r[sel0]
            for g in range(NG):
                gsel = sel0[(rr0 >> 8) == g]
                rg = r[gsel]
                if g < 8:
                    j = (rg >> 6) & 3
                    order = np.argsort(j, kind="stable")
                    gsel, rg, j = gsel[order], rg[order], j[order]
                    cnt = np.bincount(j, minlength=4)
                    is_root = np.zeros(rg.size, bool)
                    if g == 0:
                        # synthetic root entry joins block 0's stream end
                        ins = cnt[0]
                        gsel = np.insert(gsel, ins, -1)
                        rg = np.insert(rg, ins, 0)
                        j = np.insert(j, ins, 0)
                        is_root = np.insert(is_root, ins, True)
                        cnt[0] += 1
                    start = np.zeros(4, np.int64)
                    pos_in = np.arange(rg.size) - np.concatenate(
                        [[0], np.cumsum(cnt)])[:-1][j]
                    e = 0
                    for blk in range(4):
                        start[blk] = max(e, 128 * blk)
                        e = start[blk] + cnt[blk]
                    if cnt.max() > 256 or e > SLAB_A or \
                       (start[:3] + cnt[:3] > [256, 384, 512]).any():
                        raise RuntimeError("chain capacity exceeded")
                    pos = start[j] + pos_in
                    tl = pos >> 7
                    if (tl > j + 1).any() or (tl < j).any():
                        raise RuntimeError("chain incidence violated")
                    rng = np.where(tl == j, 2 * j, 2 * j + 1)
                    c2 = SLAB_A + rng * P + 2 * (rg & 63)
                else:
                    order = np.argsort(rg, kind="stable")
                    gsel, rg = gsel[order], rg[order]
                    if rg.size > SLAB_A:
                        raise RuntimeError("upper capacity exceeded")
                    pos = np.arange(rg.size)
                    tl = pos >> 7
                    is_root = np.zeros(rg.size, bool)

                p = pos & 127
                r1 = (rg >> 1) & 127
                acol = tl * P + r1
                slot = g * TPG + tl
                t_global = bb * TILES_PER_BATCH + slot

                real = ~is_root
                vals_s[t_global * P + p] = np.where(
                    is_root[:, None], root_filler[b].astype(np.float16),
                    mem_values[gsel].astype(np.float16))
                # meta1: (WA, OPA) at [slot*2], zeros for root
                meta_s[bb, p[real], slot[real] * 2] = wA[gsel[real]]
                meta_s[bb, p[real], slot[real] * 2 + 1] = opA[gsel[real]]
                # A one-hot idx (col 0..4 by tile) — skip root
                idx_s[bb, p[real], g, tl[real]] = acol[real]
                if g < 8:
                    # meta2: (WB, WC, OP2) at [160 + (g*5+tl)*3]
                    base = 160 + slot * 3
                    meta_s[bb, p[real], base[real]] = w[gsel[real], 2]
                    meta_s[bb, p[real], base[real] + 1] = w[gsel[real], 3]
                    meta_s[bb, p, base + 2] = op_dist[b, 2]
                    if is_root.any():
                        meta_s[bb, p[is_root], base[is_root] + 1] = 1.0
                    idx_s[bb, p[real], g, 5 + tl[real]] = c2[real]
                    idx_s[bb, p, g, 10 + tl] = c2 + 1
                    if is_root.any():
                        # root has no even-bin write
                        idx_s[bb, p[is_root], g, 5 + tl[is_root]] = -1
                else:
                    us = (g - 8) * TPG + tl
                    r1pm_s[bb, p, us, 0] = r1
                    r1pm_s[bb, p, us, 1] = -r1.astype(np.float32)

        in_maps.append({
            "vals": np.ascontiguousarray(
                vals_s.reshape(NVS, VB, P, F).transpose(0, 2, 1, 3)),
            "meta": meta_s,
            "r1pm": r1pm_s,
            "idxs": idx_s,
            "iota": iota_np,
        })
    return in_maps


def kernel(**inputs):
    from concourse.bass_utils import run_bass_kernel_spmd

    in_maps = _pack_inputs(**inputs)
    if "nc" not in _PROG_CACHE:
        _PROG_CACHE["nc"] = _build_program()
    nc = _PROG_CACHE["nc"]
    res = run_bass_kernel_spmd(nc, in_maps, list(range(NCORES)))
    return np.ascontiguousarray(np.concatenate(
        [res.results[c]["out"].transpose(0, 2, 1) for c in range(NCORES)],
        axis=0))


# revision 10
# speedup vs baseline: 1.2481x; 1.0898x over previous
"""DiffTreeInterpreter scatter-coalesce kernel for 8 Trainium2 cores.

Data-parallel over batch B=32: core c owns batches [4c, 4c+4). Host work
is index prep only: bucketing entries by (batch, role-group), shipping
bit-exact copies of per-entry weights; all arithmetic happens on device.

Math (see reference): with H = R/2, entry n (b, l, r, v=mem[n],
w=arg_weights[b,l]) contributes to out[b] at up to 3 bins:
  bin r>>1  with uA = op0*w0 (r even) / op1*w1 (r odd, r!=1)   [A-stream]
  bin 2r    with u2 = op2*w2  (only r < H)                     [cons even]
  bin 2r+1  with u3 = op2*w3  (only r < H)                     [cons odd]
plus out[b,1] += op2*root_filler[b]. (Pad-mask is a no-op: masked rows
are all-zero values.)

Device design (per batch):
- 16 groups of 256 roles, 5 value tiles (128 entries) each = 80 tiles.
  Lower groups g<8 chain-pack their 4 cons blocks (64 roles) into the 5
  tiles with static (tile, block) incidence T0:A T1:AB T2:BC T3:CD T4:D.
- One-hot slabs [entry-partition, cols] fp16: lower = merged [640 A-cols
  | 1024 cons-cols] built by one GPSIMD local_scatter; upper A-slabs
  [640] built per-tile on DVE (iota EQ r1 * u) or ACT (2-op tent:
  Square then Relu with per-partition bias/scale).
- Matmuls are value-stationary: out[f, bins] = v[entry,F]^T @ slab.
  PSUM holds 8 "superblock" banks [128, 512 bins] per batch; A-stream
  and cons matmuls accumulate into shared bank slices (A starts a
  slice, cons of the owning lower group finishes it), so no separate
  combine pass exists.
- Banks drain via one wide ACT copy [128,512] into paired staging, then
  DMA to out[b, F, R] (host transposes at unshard).
"""

import sys

if "/opt/trn_rl_repo" not in sys.path:
    sys.path.insert(0, "/opt/trn_rl_repo")

import numpy as np

B, L, F, R = 32, 128, 128, 4096
H = R >> 1
N = 262144
NCORES = 8
BPC = B // NCORES
P = 128

NG = 16          # role groups per batch (256 roles each)
TPG = 5          # value tiles per group
TILES_PER_BATCH = NG * TPG  # 80
NT = BPC * TILES_PER_BATCH  # 320 tiles per core
VB = 16          # value tiles per load DMA
NVS = NT // VB   # 20 value load slabs per core

SLAB_A = TPG * P        # 640
SLAB_W = SLAB_A + 8 * P  # 1664 (A cols + 8 cons ranges)

# upper-group one-hot builder assignment (groups 8..15)
DVE_G = (8, 9, 10, 11, 12, 13)
ACT_G = (14, 15)

# per-batch step schedule: upper groups first (their slabs build on
# DVE/ACT while the GPSIMD library loads / lower scatters pipeline),
# cons+drain spread across the back half
ORDER_A = (8, 9, 10, 11, 12, 13, 14, 15, 0, 1, 2, 3, 4, 5, 6, 7)
CONS_AT = {3: 2, 7: 3, 8: 4, 9: 5, 10: 6, 11: 7, 12: 0, 15: 1}
# value stream group order = first-use order of each group's tiles
STREAM_G = (8, 9, 10, 11, 12, 13, 14, 15, 0, 4, 1, 5, 2, 6, 3, 7)
SPOS = {g: i for i, g in enumerate(STREAM_G)}

_PROG_CACHE = {}


def _build_program():
    import concourse.bacc as bacc
    import concourse.mybir as mybir
    import concourse.tile as tile

    fp32 = mybir.dt.float32
    fp16 = mybir.dt.float16
    i16 = mybir.dt.int16
    MUL = mybir.AluOpType.mult
    EQ = mybir.AluOpType.is_equal
    AF = mybir.ActivationFunctionType

    nc = bacc.Bacc(None, target_bir_lowering=False)
    vals = nc.dram_tensor("vals", [NVS, P, VB, F], fp16, kind="ExternalInput")
    # fp16 meta: [0:160) = (WA, OPA) per slot (g*5+t); [160:280) = lower
    # (WB, WC, OP2) per (g, t)
    meta = nc.dram_tensor("meta", [BPC, P, 280], fp16, kind="ExternalInput")
    # fp32 (r1, -r1) per upper slot ((g-8)*5+t)
    r1pm = nc.dram_tensor("r1pm", [BPC, P, 40, 2], fp32, kind="ExternalInput")
    idxs = nc.dram_tensor("idxs", [BPC, P, NG, 16], i16, kind="ExternalInput")
    iota = nc.dram_tensor("iota", [P, P], fp16, kind="ExternalInput")
    out = nc.dram_tensor("out", [BPC, F, R], fp32, kind="ExternalOutput")

    with tile.TileContext(nc) as tc:
        with tc.tile_pool(name="cst", bufs=1) as cpool, \
             tc.tile_pool(name="meta", bufs=BPC) as mpool, \
             tc.tile_pool(name="ud", bufs=BPC) as upool, \
             tc.tile_pool(name="vload", bufs=10) as vpool, \
             tc.tile_pool(name="slab", bufs=13) as spool, \
             tc.tile_pool(name="sq", bufs=2) as qpool, \
             tc.tile_pool(name="stage", bufs=8) as gpool, \
             tc.tile_pool(name="bank", bufs=8, space="PSUM") as bpool:

            io_t = cpool.tile([P, P], fp16, tag="iota")
            nc.sync.dma_start(out=io_t[:], in_=iota[:])

            vtiles = {}

            def vload(vs):
                if vs not in vtiles:
                    vt = vpool.tile([P, VB, F], fp16, tag="v")
                    nc.sync.dma_start(out=vt[:], in_=vals[vs])
                    vtiles[vs] = vt

            # prefetch metadata for all batches + compute u products
            metas = []
            for b in range(BPC):
                m = mpool.tile([P, 280], fp16, tag="m")
                nc.sync.dma_start(out=m[:], in_=meta[b])
                rp = mpool.tile([P, 40, 2], fp32, tag="rp")
                nc.sync.dma_start(out=rp[:], in_=r1pm[b])
                x = mpool.tile([P, NG, 16], i16, tag="x")
                nc.sync.dma_start(out=x[:], in_=idxs[b])

                m1 = m[:, 0:160].rearrange("p (s c) -> p s c", c=2)
                m2 = m[:, 160:280].rearrange("p (g t c) -> p g t c", g=8, c=3)
                ud = upool.tile([P, NG, 16], fp16, tag="ud")
                # uA = WA*OPA into ud[:, :, 0:5]
                nc.vector.tensor_tensor(
                    out=ud[:, :, 0:5],
                    in0=m1[:, :, 0].rearrange("p (g t) -> p g t", t=TPG),
                    in1=m1[:, :, 1].rearrange("p (g t) -> p g t", t=TPG),
                    op=MUL)
                # u2 = WB*OP2 into ud[:, 0:8, 5:10]; u3 = WC*OP2 into [10:15]
                nc.vector.tensor_tensor(
                    out=ud[:, 0:8, 5:10], in0=m2[:, :, :, 0],
                    in1=m2[:, :, :, 2], op=MUL)
                nc.vector.tensor_tensor(
                    out=ud[:, 0:8, 10:15], in0=m2[:, :, :, 1],
                    in1=m2[:, :, :, 2], op=MUL)
                # fp32 u (and -u) for upper-slot scalar operands
                u1f = upool.tile([P, 40], fp32, tag="u1f")
                nc.vector.tensor_tensor(
                    out=u1f[:], in0=m1[:, 40:80, 0], in1=m1[:, 40:80, 1],
                    op=MUL)
                ngu = upool.tile([P, 40], fp32, tag="ngu")
                nc.vector.tensor_scalar(
                    out=ngu[:], in0=u1f[:], scalar1=-1.0, scalar2=None,
                    op0=MUL)
                metas.append((ud, x, u1f, ngu, rp))
                if b == 0:
                    vload(0)
                    vload(1)

            for b in range(BPC):
                ud, x, u1f, ngu, rp = metas[b]

                def vtile(g, tl):
                    t = b * TILES_PER_BATCH + SPOS[g] * TPG + tl
                    vload(t // VB)
                    if (t // VB) + 1 < NVS:
                        vload(t // VB + 1)
                    return vtiles[t // VB][:, t % VB, :]

                banks = [None] * 8
                slabs = [None] * NG
                stages = [None] * 4

                def drain(k):
                    # pair (k, k^1) shares a staging tile; DMA on 2nd drain
                    pair = k >> 1
                    if stages[pair] is None:
                        stages[pair] = gpool.tile([P, 1024], fp32, tag="st", name="st")
                    st = stages[pair]
                    half = (k & 1) * 512
                    nc.scalar.copy(out=st[:, half:half + 512],
                                   in_=banks[k][:])
                    if k & 1:
                        nc.sync.dma_start(
                            out=out[b, :, (pair * 1024):(pair * 1024 + 1024)],
                            in_=st[:])

                def build_slab(g):
                    if slabs[g] is not None:
                        return slabs[g]
                    sl = spool.tile([P, SLAB_W], fp16, tag="sl", name="sl")
                    slabs[g] = sl
                    if g < 8:
                        nc.gpsimd.local_scatter(
                            out_ap=sl[:], data_ap=ud[:, g, :],
                            idxs_ap=x[:, g, :],
                            channels=P, num_elems=SLAB_W, num_idxs=16)
                    elif g in DVE_G:
                        for tl in range(TPG):
                            s = (g - 8) * TPG + tl
                            nc.vector.tensor_scalar(
                                out=sl[:, tl * P:(tl + 1) * P], in0=io_t[:],
                                scalar1=rp[:, s, 0:1],
                                scalar2=u1f[:, s:s + 1],
                                op0=EQ, op1=MUL)
                    else:
                        for tl in range(TPG):
                            s = (g - 8) * TPG + tl
                            sq = qpool.tile([P, P], fp16, tag="sq", name="sq")
                            nc.scalar.activation(
                                out=sq[:], in_=io_t[:], func=AF.Square,
                                bias=rp[:, s, 1:2], scale=1.0)
                            nc.scalar.activation(
                                out=sl[:, tl * P:(tl + 1) * P], in_=sq[:],
                                func=AF.Relu, bias=u1f[:, s:s + 1],
                                scale=ngu[:, s:s + 1])
                    return sl

                def cons(cg, bank_fresh):
                    # group cg's cons into bank cg; block j gets
                    # (T_j: range 2j, T_{j+1}: range 2j+1). PSUM
                    # start=True clears has_written for the WHOLE bank,
                    # so only a bank's first-ever matmul may set it;
                    # start=False overwrites where the bit is clear and
                    # accumulates where set.
                    csl = build_slab(cg)
                    for tl in range(TPG):
                        v = vtile(cg, tl)
                        if tl >= 1:  # tile ends block tl-1
                            rng = SLAB_A + (2 * (tl - 1) + 1) * P
                            nc.tensor.matmul(
                                out=banks[cg][:, (tl - 1) * P:tl * P],
                                lhsT=v, rhs=csl[:, rng:rng + P],
                                start=False, stop=True,
                                skip_group_check=True)
                        if tl <= 3:  # tile starts block tl
                            rng = SLAB_A + (2 * tl) * P
                            nc.tensor.matmul(
                                out=banks[cg][:, tl * P:(tl + 1) * P],
                                lhsT=v, rhs=csl[:, rng:rng + P],
                                start=(bank_fresh and tl == 0),
                                stop=False, skip_group_check=True)

                for step, g in enumerate(ORDER_A):
                    sl = build_slab(g)

                    # --- A-stream matmuls: bank g>>2, slice g&3 ---
                    bk = g >> 2
                    first = banks[bk] is None
                    if first:
                        banks[bk] = bpool.tile([P, 512], fp32, tag="bk", name="bk")
                    sli = (g & 3) * P
                    for tl in range(TPG):
                        v = vtile(g, tl)
                        nc.tensor.matmul(
                            out=banks[bk][:, sli:sli + P], lhsT=v,
                            rhs=sl[:, tl * P:(tl + 1) * P],
                            start=(first and tl == 0), stop=False,
                            skip_group_check=True)

                    # --- cons + drain per schedule ---
                    if step in CONS_AT:
                        cg = CONS_AT[step]
                        fresh = banks[cg] is None
                        if fresh:
                            banks[cg] = bpool.tile([P, 512], fp32, tag="bk", name="bk")
                        cons(cg, bank_fresh=fresh)
                        drain(cg)

    nc.compile()
    return nc


def _pack_inputs(mem_values, arg_weights, root_filler, op_dist,
                 batch_idx, slot_idx, role_idx):
    """Host-side sharding/packing: index selection and copies only."""
    mem_values = np.ascontiguousarray(mem_values, dtype=np.float32)
    arg_weights = np.asarray(arg_weights, dtype=np.float32)
    root_filler = np.asarray(root_filler, dtype=np.float32)
    op_dist = np.asarray(op_dist, dtype=np.float32)
    batch_idx = np.asarray(batch_idx, dtype=np.int64)
    slot_idx = np.asarray(slot_idx, dtype=np.int64)
    role_idx = np.asarray(role_idx, dtype=np.int64)

    w = arg_weights[batch_idx, slot_idx]  # [N, 4] gathered copies
    r = role_idx
    even = (r & 1) == 0
    wA = np.where(even, w[:, 0], np.where(r != 1, w[:, 1], 0.0))
    opA = np.where(even, op_dist[batch_idx, 0], op_dist[batch_idx, 1])

    iota_np = np.broadcast_to(
        np.arange(P, dtype=np.float16), (P, P)).copy()

    in_maps = []
    for c in range(NCORES):
        vals_s = np.zeros((NT * P, F), np.float16)
        meta_s = np.zeros((BPC, P, 280), np.float16)
        r1pm_s = np.zeros((BPC, P, 40, 2), np.float32)
        r1pm_s[:, :, :, 0] = -1.0
        r1pm_s[:, :, :, 1] = 1.0
        idx_s = np.full((BPC, P, NG, 16), -1, np.int16)

        for bb in range(BPC):
            b = c * BPC + bb
            sel0 = np.nonzero(batch_idx == b)[0]
            rr0 = r[sel0]
            for g in range(NG):
                gsel = sel0[(rr0 >> 8) == g]
                rg = r[gsel]
                if g < 8:
                    j = (rg >> 6) & 3
                    order = np.argsort(j, kind="stable")
                    gsel, rg, j = gsel[order], rg[order], j[order]
                    cnt = np.bincount(j, minlength=4)
                    is_root = np.zeros(rg.size, bool)
                    if g == 0:
                        # synthetic root entry joins block 0's stream end
                        ins = cnt[0]
                        gsel = np.insert(gsel, ins, -1)
                        rg = np.insert(rg, ins, 0)
                        j = np.insert(j, ins, 0)
                        is_root = np.insert(is_root, ins, True)
                        cnt[0] += 1
                    start = np.zeros(4, np.int64)
                    pos_in = np.arange(rg.size) - np.concatenate(
                        [[0], np.cumsum(cnt)])[:-1][j]
                    e = 0
                    for blk in range(4):
                        start[blk] = max(e, 128 * blk)
                        e = start[blk] + cnt[blk]
                    if cnt.max() > 256 or e > SLAB_A or \
                       (start[:3] + cnt[:3] > [256, 384, 512]).any():
                        raise RuntimeError("chain capacity exceeded")
                    pos = start[j] + pos_in
                    tl = pos >> 7
                    if (tl > j + 1).any() or (tl < j).any():
                        raise RuntimeError("chain incidence violated")
                    rng = np.where(tl == j, 2 * j, 2 * j + 1)
                    c2 = SLAB_A + rng * P + 2 * (rg & 63)
                else:
                    order = np.argsort(rg, kind="stable")
                    gsel, rg = gsel[order], rg[order]
                    if rg.size > SLAB_A:
                        raise RuntimeError("upper capacity exceeded")
                    pos = np.arange(rg.size)
                    tl = pos >> 7
                    is_root = np.zeros(rg.size, bool)

                p = pos & 127
                r1 = (rg >> 1) & 127
                acol = tl * P + r1
                slot = g * TPG + tl
                t_global = bb * TILES_PER_BATCH + SPOS[g] * TPG + tl

                real = ~is_root
                vals_s[t_global * P + p] = np.where(
                    is_root[:, None], root_filler[b].astype(np.float16),
                    mem_values[gsel].astype(np.float16))
                # meta1: (WA, OPA) at [slot*2], zeros for root
                meta_s[bb, p[real], slot[real] * 2] = wA[gsel[real]]
                meta_s[bb, p[real], slot[real] * 2 + 1] = opA[gsel[real]]
                # A one-hot idx (col 0..4 by tile) — skip root
                idx_s[bb, p[real], g, tl[real]] = acol[real]
                if g < 8:
                    # meta2: (WB, WC, OP2) at [160 + (g*5+tl)*3]
                    base = 160 + slot * 3
                    meta_s[bb, p[real], base[real]] = w[gsel[real], 2]
                    meta_s[bb, p[real], base[real] + 1] = w[gsel[real], 3]
                    meta_s[bb, p, base + 2] = op_dist[b, 2]
                    if is_root.any():
                        meta_s[bb, p[is_root], base[is_root] + 1] = 1.0
                    idx_s[bb, p[real], g, 5 + tl[real]] = c2[real]
                    idx_s[bb, p, g, 10 + tl] = c2 + 1
                    if is_root.any():
                        # root has no even-bin write
                        idx_s[bb, p[is_root], g, 5 + tl[is_root]] = -1
                else:
                    us = (g - 8) * TPG + tl
                    r1pm_s[bb, p, us, 0] = r1
                    r1pm_s[bb, p, us, 1] = -r1.astype(np.float32)

        in_maps.append({
            "vals": np.ascontiguousarray(
                vals_s.reshape(NVS, VB, P, F).transpose(0, 2, 1, 3)),
            "meta": meta_s,
            "r1pm": r1pm_s,
            "idxs": idx_s,
            "iota": iota_np,
        })
    return in_maps


def kernel(**inputs):
    from concourse.bass_utils import run_bass_kernel_spmd

    in_maps = _pack_inputs(**inputs)
    if "nc" not in _PROG_CACHE:
        _PROG_CACHE["nc"] = _build_program()
    nc = _PROG_CACHE["nc"]
    res = run_bass_kernel_spmd(nc, in_maps, list(range(NCORES)))
    return np.ascontiguousarray(np.concatenate(
        [res.results[c]["out"].transpose(0, 2, 1) for c in range(NCORES)],
        axis=0))
